# revision 3
# baseline (speedup 1.0000x reference)
"""Trainium2 Bass kernel for nn_AttentionLayer (diagonal-projection attention).

Math (per batch b, head h):
  g_h   = diag(W_Q[h]) * diag(W_K[h])                       # [D]
  S_h   = (X_Q[b] * g_h) @ X_K[b].T / sqrt(D)               # [Lq, Lk]
  E_h   = exp(S_h) * mask[b]                                # multiplicative mask
                                                            # (scores are tiny, no max-sub needed)
  l_h   = E_h.sum(-1)                                       # [Lq]
  out  += (E_h / l_h[:, None]) @ X_V[b] @ diag(dv_h) @ O_h  # [Lq, F]

Device computes, per core (b = core//4, two heads h0, h1 = 2*(core%4)(+1)):
  Y_h^T[f', q] = sum_k W_h[k, f'] * E_h^T[k, q]   with W_h = X_V[b] @ diag(dv_h) @ O_h
  l_h[q]       = sum_k E_h^T[k, q]                (ones-matmul on PE)
E^T is produced directly in [k, q] layout by computing transposed scores, so no
on-chip transpose is ever needed.  Host folds g into X_Q, precomputes W_h,
pre-transposes inputs, and finishes with out = sum_h Y_h / l_h (+ gather).
"""

import numpy as np
import ml_dtypes

B, H, L, D = 2, 8, 2048, 128
NCORES = 8
HEADS_PER_CORE = H * B // NCORES  # 2
KT = L // 128  # 16 k-tiles
QH = 2         # q halves
QHW = L // QH  # 1024
SCALE = 1.0 / np.sqrt(np.float32(D))

_NC = None


def build_nc():
    import concourse.bass as bass  # noqa: F401
    import concourse.mybir as mybir
    import concourse.tile as tile
    from concourse import bacc

    bf16 = mybir.dt.bfloat16
    f32 = mybir.dt.float32

    nc = bacc.Bacc("TRN2", target_bir_lowering=False, debug=False)

    # DRAM parameters (per-core shards)
    xqg_d = nc.dram_tensor("xqg", [HEADS_PER_CORE, 128, L], bf16, kind="ExternalInput").ap()
    xkt_d = nc.dram_tensor("xkt", [128, L], bf16, kind="ExternalInput").ap()
    w_d = nc.dram_tensor("w", [HEADS_PER_CORE, L, 128], bf16, kind="ExternalInput").ap()
    maskt_d = nc.dram_tensor("maskt", [L, L], bf16, kind="ExternalInput").ap()
    y_d = nc.dram_tensor("y", [HEADS_PER_CORE, 128, L], f32, kind="ExternalOutput").ap()
    l_d = nc.dram_tensor("l", [HEADS_PER_CORE, L], f32, kind="ExternalOutput").ap()

    with tile.TileContext(nc) as tc:
        with (
            tc.tile_pool(name="singles", bufs=1) as singles,
            tc.tile_pool(name="maskp", bufs=2) as maskp,
            tc.tile_pool(name="ep", bufs=3) as ep,
            tc.tile_pool(name="emp", bufs=3) as emp,
            tc.tile_pool(name="ysb", bufs=2) as ysbp,
            tc.tile_pool(name="spsum", bufs=2, space="PSUM") as spsum,
            tc.tile_pool(name="ypsum", bufs=1, space="PSUM") as ypsum,
            tc.tile_pool(name="lpsum", bufs=1, space="PSUM") as lpsum,
        ):
            # Load all small inputs once.
            xqg = singles.tile([128, HEADS_PER_CORE, L], bf16)
            nc.sync.dma_start(out=xqg, in_=xqg_d.rearrange("h p q -> p h q"))
            xkt = singles.tile([128, L], bf16)
            nc.sync.dma_start(out=xkt, in_=xkt_d)
            w = singles.tile([128, HEADS_PER_CORE, KT, 128], bf16)
            nc.sync.dma_start(out=w, in_=w_d.rearrange("h (kt p) f -> p h kt f", p=128))
            ones = singles.tile([128, 1], bf16)
            nc.vector.memset(ones, 1.0)

            maskt_r = maskt_d.rearrange("(kt p) q -> p kt q", p=128)

            for qh in range(QH):
                qs = qh * QHW
                mask_blk = maskp.tile([128, KT, QHW], bf16)
                nc.sync.dma_start(out=mask_blk, in_=maskt_r[:, :, qs:qs + QHW])
                for h in range(HEADS_PER_CORE):
                    y_ps = ypsum.tile([128, QHW], f32)
                    l_ps = lpsum.tile([1, QHW], f32)
                    for kt in range(KT):
                        s_ps = spsum.tile([128, QHW], f32)
                        for c in range(QHW // 512):
                            nc.tensor.matmul(
                                s_ps[:, c * 512:(c + 1) * 512],
                                xkt[:, kt * 128:(kt + 1) * 128],
                                xqg[:, h, qs + c * 512: qs + (c + 1) * 512],
                                start=True, stop=True,
                            )
                        e_t = ep.tile([128, QHW], bf16)
                        nc.scalar.activation(
                            e_t, s_ps, mybir.ActivationFunctionType.Exp,
                            bias=0.0, scale=float(SCALE),
                        )
                        em_t = emp.tile([128, QHW], bf16)
                        nc.vector.tensor_mul(em_t, e_t, mask_blk[:, kt, :])
                        for c in range(QHW // 512):
                            sl = slice(c * 512, (c + 1) * 512)
                            nc.tensor.matmul(
                                y_ps[:, sl], w[:, h, kt, :], em_t[:, sl],
                                start=(kt == 0), stop=(kt == KT - 1),
                            )
                            nc.tensor.matmul(
                                l_ps[:, sl], ones, em_t[:, sl],
                                start=(kt == 0), stop=(kt == KT - 1),
                            )
                    y_sb = ysbp.tile([128, QHW], f32)
                    nc.vector.tensor_copy(y_sb, y_ps)
                    nc.sync.dma_start(out=y_d[h, :, qs:qs + QHW], in_=y_sb)
                    l_sb = ysbp.tile([1, QHW], f32)
                    nc.vector.tensor_copy(l_sb, l_ps)
                    nc.sync.dma_start(out=l_d[h:h + 1, qs:qs + QHW], in_=l_sb)
    nc.compile()
    return nc


def get_nc():
    global _NC
    if _NC is None:
        _NC = build_nc()
    return _NC


def host_prep(X_Q, X_K, X_V, mask, W_Q, W_K, W_V, O):
    """Build per-core input shards (numpy, bf16)."""
    bf = ml_dtypes.bfloat16
    dq = np.einsum("hdd->hd", np.asarray(W_Q, np.float32))
    dk = np.einsum("hdd->hd", np.asarray(W_K, np.float32))
    dv = np.einsum("hff->hf", np.asarray(W_V, np.float32))
    g = dq * dk  # [H, D]
    X_Q = np.asarray(X_Q, np.float32)
    X_K = np.asarray(X_K, np.float32)
    X_V = np.asarray(X_V, np.float32)
    O = np.asarray(O, np.float32).reshape(H, D, D)  # [h, f, f']
    mask = np.asarray(mask)

    # W_h = X_V[b] @ diag(dv_h) @ O_h  -> [B, H, L, F']
    Wf = np.einsum("blf,hf,hfe->bhle", X_V, dv, O).astype(bf)
    # XQg^T: [B, H, D, L]
    xqgT = np.einsum("bld,hd->bhdl", X_Q, g).astype(bf)
    xkT = X_K.transpose(0, 2, 1).astype(bf)          # [B, D, L]
    maskT = mask[:, 0].transpose(0, 2, 1).astype(bf)  # [B, Lk, Lq]

    in_maps = []
    for c in range(NCORES):
        b = c // 4
        h0 = 2 * (c % 4)
        in_maps.append({
            "xqg": np.ascontiguousarray(xqgT[b, h0:h0 + 2]),
            "xkt": np.ascontiguousarray(xkT[b]),
            "w": np.ascontiguousarray(Wf[b, h0:h0 + 2]),
            "maskt": np.ascontiguousarray(maskT[b]),
        })
    return in_maps


def host_combine(results):
    """results: list of 8 dicts with 'y' [2,128,L] f32 and 'l' [2,L] f32."""
    out = np.zeros((B, L, D), np.float32)
    for c, r in enumerate(results):
        b = c // 4
        y = r["y"]  # [2, 128, L]
        l = r["l"]  # [2, L]
        for i in range(HEADS_PER_CORE):
            out[b] += (y[i] / l[i][None, :]).T
    return out


def kernel(X_Q, X_K, X_V, mask, W_Q, W_K, W_V, O, _trace=False):
    from concourse.bass_utils import run_bass_kernel_spmd

    nc = get_nc()
    in_maps = host_prep(X_Q, X_K, X_V, mask, W_Q, W_K, W_V, O)
    res = run_bass_kernel_spmd(nc, in_maps, core_ids=list(range(NCORES)), trace=_trace)
    out = host_combine(res.results)
    if _trace:
        return out, res
    return out
